# revision 2
# baseline (speedup 1.0000x reference)
"""Weighted per-class dice loss on 8 trn2 NeuronCores (batch-sharded), v2.

Per core (one batch element b), pixels viewed as [128, fcol]:
  - SWDGE (gpsimd) casting DMAs bring pred chunks in as bf16 [128, 19, CH].
  - DVE computes, per chunk, two 2x-mode tensor_tensor ops into interleaved
    [P, CH/QC, 20, QC] buffers (QC pixel columns per matmul):
      PW[:, t, c, q]  = pred_c * W       (in1 = W broadcast over classes)
      MM[:, t, c, q]  = (L == c)          (vs a tiny broadcast CLS constant)
    plus a tiny copy of W into PW group 19; MM group 19 = ones.
  - PE accumulates, for each t, one matmul over QC pixel columns
      out[m, n] += sum_p PW[p, t, m] * MM[p, t, n]   (m = g*QC + q)
    into a single [80, 80] PSUM accumulation group:
      out[c, c]   -> inter[c]   (pred_c * W summed where L == c)
      out[c, 19]  -> psum[c]    (pred_c * W summed everywhere)
      out[19, c]  -> tsum[c]    (W summed where L == c)
  - Host folds the QC column-phases + 8 cores and applies the dice formula.
"""

import numpy as np

import concourse.bass as bass
from concourse import mybir
from concourse.bass_utils import run_bass_kernel_spmd

C = 19
G = 20            # 19 classes + 1 (W / ones group)
P = 128
FCOL_FULL = 4096
CH_FULL = 512     # chunk width (pixel columns per chunk)
QC = 4            # pixel columns folded into one matmul
SMOOTH = 1.0

F = mybir.dt.float32
BF = mybir.dt.bfloat16

mult = mybir.AluOpType.mult
is_eq = mybir.AluOpType.is_equal


def chunk_widths(fcol: int, ch: int) -> list[int]:
    """Taper first/last chunks to shrink pipeline fill/drain time."""
    if fcol % ch or fcol // ch < 4 or ch % (2 * QC):
        return [ch] * max(fcol // ch, 1) if fcol >= ch else [fcol]
    h, q = ch // 2, ch // 4
    mid = (fcol - 2 * h - h - 2 * q) // ch
    return [h, h] + [ch] * mid + [h, q, q]


def build_nc(fcol: int = FCOL_FULL, ch: int = CH_FULL) -> bass.Bass:
    cws = chunk_widths(fcol, ch)
    nch = len(cws)
    offs = [sum(cws[:k]) for k in range(nch)]
    assert sum(cws) == fcol and all(c % QC == 0 for c in cws)
    tpc = ch // QC  # matmuls per full-size chunk (buffer capacity)
    head = cws[0] + cws[1]  # columns covered by the early W/L pieces

    nc = bass.Bass()
    pred = nc.dram_tensor("pred", [C, P, fcol], F, kind="ExternalInput")
    tgt = nc.dram_tensor("target", [2, P, fcol], F, kind="ExternalInput")
    assert nch >= 3
    partials = nc.dram_tensor(
        "partials", [G * QC, 2 * G * QC], F, kind="ExternalOutput")

    from contextlib import ExitStack

    _es = ExitStack()
    with _es:
        def sb(name, shape, dt):
            return _es.enter_context(nc.sbuf_tensor(name, shape, dt))

        NPB = 3  # pred chunk buffers
        lb = sb("lb", [P, fcol], BF)
        wb = sb("wb", [P, fcol], BF)
        cls = sb("cls", [P, C, QC], BF)
        predc = [sb(f"predc{i}", [P, C, ch], BF) for i in range(NPB)]
        pww = [sb(f"pww{i}", [P, tpc, G, QC], BF) for i in range(2)]
        mmo = [sb(f"mmo{i}", [P, tpc, G, QC], BF) for i in range(2)]

        # chunks 6/7/8 get their masks prebuilt by gpsimd via per-class
        # tensor_scalar is_equal ops (plain tensor_tensor is rejected by the
        # real TRN2 ISA on the Pool engine; tensor_scalar is accepted)
        pm = ((6, 7, 8) if nch >= 10 and cws[6] == cws[7] == ch else ())
        mmo_pm = {k: sb(f"mmopm{j}", [P, cws[k] // QC, G, QC], BF)
                  for j, k in enumerate(pm)}
        outsb = sb("outsb", [G * QC, 2 * G * QC], F)
        ps = _es.enter_context(nc.psum_tensor("ps", [G * QC, G * QC], F))
        ps2 = _es.enter_context(nc.psum_tensor("ps2", [G * QC, G * QC], F))

        def sem(name):
            return _es.enter_context(nc.semaphore(name))

        l_sem = sem("l_sem")
        lrest_sem = sem("lrest_sem")
        w_sem = sem("w_sem")
        wrest_sem = sem("wrest_sem")
        c0b_sem = sem("c0b_sem")
        done_sem = sem("done_sem")
        cls_sem = sem("cls_sem")
        ones_sem = sem("ones_sem")
        pdma_sems = [sem(f"pdma_sem{i}") for i in range(NPB)]
        vsem = sem("vsem")
        asem = sem("asem")
        pe_sem = sem("pe_sem")
        pmask_sem = sem("pmask_sem")

        block = _es.enter_context(nc.Block())

        @block.gpsimd
        def _(g: bass.BassEngine):
            # tiny class-id constant [P, C, QC]; value = class index
            g.iota(cls[:], pattern=[[1, C], [0, QC]], base=0,
                   channel_multiplier=0,
                   allow_small_or_imprecise_dtypes=True).then_inc(cls_sem, 1)
            # L/W head pieces first (gate chunk-0/1 compute), rests after c1
            g.dma_start(out=lb[:, 0:head], in_=tgt[0][:, 0:head]).then_inc(
                l_sem, 16)
            g.dma_start(out=wb[:, 0:head], in_=tgt[1][:, 0:head]).then_inc(
                w_sem, 16)

            def chunk_dma(k):
                i, cw, o = k % NPB, cws[k], offs[k]
                src = pred.ap()[:, :, o:o + cw].transpose([1, 0, 2])
                g.dma_start(out=predc[i][:, :, 0:cw], in_=src).then_inc(
                    pdma_sems[i], 16)

            # chunk 0 in two class-halves so its pw can start sooner
            cw0 = cws[0]
            src0 = pred.ap()[0:10, :, 0:cw0].transpose([1, 0, 2])
            g.dma_start(out=predc[0][:, 0:10, 0:cw0], in_=src0).then_inc(
                pdma_sems[0], 16)
            src0b = pred.ap()[10:C, :, 0:cw0].transpose([1, 0, 2])
            g.dma_start(out=predc[0][:, 10:C, 0:cw0], in_=src0b).then_inc(
                c0b_sem, 16)
            g.memset(mmo[0][:, :, C:G, :], 1.0).then_inc(ones_sem, 1)
            chunk_dma(1)
            g.memset(mmo[1][:, :, C:G, :], 1.0).then_inc(ones_sem, 1)
            g.dma_start(out=lb[:, head:], in_=tgt[0][:, head:]).then_inc(
                lrest_sem, 16)
            g.dma_start(out=wb[:, head:], in_=tgt[1][:, head:]).then_inc(
                wrest_sem, 16)
            chunk_dma(2)

            pmops = [(kp, c) for kp in pm for c in range(C)]

            def pm_op(kp, c):
                t_k = cws[kp] // QC
                o = offs[kp]
                lsl = lb[:, o:o + cws[kp]].rearrange(
                    "p (t q) -> p t q", q=QC).unsqueeze(2)
                op = g.tensor_scalar(
                    out=mmo_pm[kp][:, 0:t_k, c:c + 1, :], in0=lsl,
                    scalar1=float(c), scalar2=None, op0=is_eq)
                if c == C - 1:
                    op.then_inc(pmask_sem, 1)

            if pm:
                for kp in pm:
                    g.memset(mmo_pm[kp][:, :, C:G, :], 1.0)
                g.wait_ge(lrest_sem, 16)
            qi = 0
            for k in range(3, nch):
                # weave premask ops ahead of each gated wait so the Pool is
                # never the reason a pred DMA stalls
                nbatch = min(len(pmops) - qi, 8)
                for _ in range(nbatch):
                    pm_op(*pmops[qi])
                    qi += 1
                g.wait_ge(vsem, k - 2)
                chunk_dma(k)
            while qi < len(pmops):
                pm_op(*pmops[qi])
                qi += 1

        @block.vector
        def _(v: bass.BassEngine):
            v.wait_ge(cls_sem, 1)
            v.wait_ge(l_sem, 16)
            for k in range(nch):
                i, ib = k % NPB, k % 2
                cw, o = cws[k], offs[k]
                t_k = cw // QC
                if k == 2:
                    v.wait_ge(lrest_sem, 16)
                    v.wait_ge(wrest_sem, 16)
                if k >= 2:
                    v.wait_ge(pe_sem, k - 1)
                # [P, t_k, 1, QC] views of this chunk's W / L columns
                wslice = wb[:, o:o + cw].rearrange(
                    "p (t q) -> p t q", q=QC).unsqueeze(2)
                lslice = lb[:, o:o + cw].rearrange(
                    "p (t q) -> p t q", q=QC).unsqueeze(2)
                pview = predc[i].ap()[:, :, 0:cw].rearrange(
                    "p c (t q) -> p t c q", q=QC)
                # mask first: it does not depend on this chunk's pred DMA
                if k not in mmo_pm:
                    v.tensor_tensor(
                        out=mmo[ib][:, 0:t_k, 0:C, :],
                        in0=lslice.broadcast_to([P, t_k, C, QC]),
                        in1=cls.ap().unsqueeze(1).broadcast_to(
                            [P, t_k, C, QC]),
                        op=is_eq,
                    )
                if k == 0:
                    v.wait_ge(w_sem, 16)
                v.wait_ge(pdma_sems[i], 16 * (k // NPB + 1))
                if k == 0:
                    v.tensor_tensor(
                        out=pww[0][:, 0:t_k, 0:10, :],
                        in0=predc[0].ap()[:, 0:10, 0:cw].rearrange(
                            "p c (t q) -> p t c q", q=QC),
                        in1=wslice.broadcast_to([P, t_k, 10, QC]), op=mult,
                    )
                    v.wait_ge(c0b_sem, 16)
                    v.tensor_tensor(
                        out=pww[0][:, 0:t_k, 10:C, :],
                        in0=predc[0].ap()[:, 10:C, 0:cw].rearrange(
                            "p c (t q) -> p t c q", q=QC),
                        in1=wslice.broadcast_to([P, t_k, C - 10, QC]),
                        op=mult,
                    ).then_inc(vsem, 1)
                else:
                    v.tensor_tensor(
                        out=pww[ib][:, 0:t_k, 0:C, :], in0=pview,
                        in1=wslice.broadcast_to([P, t_k, C, QC]), op=mult,
                    ).then_inc(vsem, 1)
            v.wait_ge(pe_sem, nch - 1)
            v.tensor_copy(out=outsb[:, 0:G * QC], in_=ps[:]).then_inc(vsem, 1)
            v.wait_ge(pe_sem, nch)
            v.tensor_copy(out=outsb[:, G * QC:], in_=ps2[:]).then_inc(vsem, 1)

        @block.scalar
        def _(a: bass.BassEngine):
            # the per-chunk W column copies (pww group 19) run on the
            # otherwise-idle Activation engine
            a.wait_ge(w_sem, 16)
            for k in range(nch):
                ib, cw, o = k % 2, cws[k], offs[k]
                t_k = cw // QC
                if k == 2:
                    a.wait_ge(wrest_sem, 16)
                if k >= 2:
                    a.wait_ge(pe_sem, k - 1)
                wslice = wb[:, o:o + cw].rearrange(
                    "p (t q) -> p t q", q=QC).unsqueeze(2)
                a.copy(out=pww[ib][:, 0:t_k, C:G, :],
                       in_=wslice).then_inc(asem, 1)

        @block.tensor
        def _(t: bass.BassEngine):
            t.wait_ge(ones_sem, 2)
            for k in range(nch):
                i = k % 2
                t_k = cws[k] // QC  # noqa: buffer parity for pww/mmo
                t.wait_ge(vsem, k + 1)
                t.wait_ge(asem, k + 1)
                if k in mmo_pm:
                    t.wait_ge(pmask_sem, 1 + pm.index(k))
                rhs = mmo_pm.get(k, mmo[i])
                last = k == nch - 1
                bank = ps2 if last else ps
                for j in range(t_k):
                    mm = t.matmul(
                        bank[:],
                        pww[i][:, j, :, :],
                        rhs[:, j, :, :],
                        start=(j == 0 and k in (0, nch - 1)),
                        stop=(j == t_k - 1 and k in (nch - 2, nch - 1)),
                    )
                mm.then_inc(pe_sem, 1)

        @block.sync
        def _(s: bass.BassEngine):
            s.wait_ge(vsem, nch + 1)
            s.dma_start(
                out=partials[:, 0:G * QC], in_=outsb[:, 0:G * QC]
            ).then_inc(done_sem, 16)
            s.wait_ge(vsem, nch + 2)
            s.dma_start(
                out=partials[:, G * QC:], in_=outsb[:, G * QC:]
            ).then_inc(done_sem, 16)

    return nc


def _combine(parts: np.ndarray) -> np.ndarray:
    # parts: [ncores, G*QC * 2*G*QC] f32 — two PSUM banks side by side
    tot = parts.astype(np.float64).sum(axis=0).reshape(G * QC, 2, G * QC)
    tot = tot.sum(axis=1).reshape(G, QC, G, QC)
    M = np.einsum("aqbq->ab", tot)
    inter = np.diagonal(M)[:C]
    psum = M[:C, C]
    tsum = M[C, :C]
    dice = (2.0 * inter + SMOOTH) / (psum + tsum + SMOOTH)
    loss = np.sum(1.0 - dice) / C
    return np.asarray(loss, dtype=np.float32)


def kernel(pred: np.ndarray, target: np.ndarray) -> np.ndarray:
    pred = np.asarray(pred, dtype=np.float32)
    target = np.asarray(target, dtype=np.float32)
    B, C_, H, Wd = pred.shape
    fcol = H * Wd // P
    pred_r = np.ascontiguousarray(pred.reshape(B, C_, P, fcol))
    tgt_r = np.ascontiguousarray(target.reshape(B, 2, P, fcol))

    nc = build_nc(fcol)
    in_maps = [{"pred": pred_r[i], "target": tgt_r[i]} for i in range(B)]
    res = run_bass_kernel_spmd(nc, in_maps, list(range(B))).results
    parts = np.stack([r["partials"].reshape(-1) for r in res])
    return _combine(parts)


# revision 3
# speedup vs baseline: 1.0208x; 1.0208x over previous
"""Weighted per-class dice loss on 8 trn2 NeuronCores (batch-sharded), v2.

Per core (one batch element b), pixels viewed as [128, fcol]:
  - SWDGE (gpsimd) casting DMAs bring pred chunks in as bf16 [128, 19, CH].
  - DVE computes, per chunk, two 2x-mode tensor_tensor ops into interleaved
    [P, CH/QC, 20, QC] buffers (QC pixel columns per matmul):
      PW[:, t, c, q]  = pred_c * W       (in1 = W broadcast over classes)
      MM[:, t, c, q]  = (L == c)          (vs a tiny broadcast CLS constant)
    plus a tiny copy of W into PW group 19; MM group 19 = ones.
  - PE accumulates, for each t, one matmul over QC pixel columns
      out[m, n] += sum_p PW[p, t, m] * MM[p, t, n]   (m = g*QC + q)
    into a single [80, 80] PSUM accumulation group:
      out[c, c]   -> inter[c]   (pred_c * W summed where L == c)
      out[c, 19]  -> psum[c]    (pred_c * W summed everywhere)
      out[19, c]  -> tsum[c]    (W summed where L == c)
  - Host folds the QC column-phases + 8 cores and applies the dice formula.
"""

import numpy as np

import concourse.bass as bass
from concourse import mybir
from concourse.bass_utils import run_bass_kernel_spmd

C = 19
G = 20            # 19 classes + 1 (W / ones group)
P = 128
FCOL_FULL = 4096
CH_FULL = 512     # chunk width (pixel columns per chunk)
QC = 4            # pixel columns folded into one matmul
SMOOTH = 1.0

F = mybir.dt.float32
BF = mybir.dt.bfloat16

mult = mybir.AluOpType.mult
is_eq = mybir.AluOpType.is_equal


def chunk_widths(fcol: int, ch: int) -> list[int]:
    """Taper first/last chunks to shrink pipeline fill/drain time."""
    if fcol % ch or ch % (2 * QC):
        return [ch] * max(fcol // ch, 1) if fcol >= ch else [fcol]
    h, q = ch // 2, ch // 4
    if fcol // ch >= 4:
        mid = (fcol - 2 * h - h - 2 * q) // ch
        return [h, h] + [ch] * mid + [h, q, q]
    return [ch] * (fcol // ch)


def build_nc(fcol: int = FCOL_FULL, ch: int = CH_FULL) -> bass.Bass:
    cws = chunk_widths(fcol, ch)
    nch = len(cws)
    offs = [sum(cws[:k]) for k in range(nch)]
    assert sum(cws) == fcol and all(c % QC == 0 for c in cws)
    tpc = ch // QC  # matmuls per full-size chunk (buffer capacity)
    head = cws[0] + cws[1]  # columns covered by the early W/L pieces

    nc = bass.Bass()
    pred = nc.dram_tensor("pred", [C, P, fcol], F, kind="ExternalInput")
    tgt = nc.dram_tensor("target", [2, P, fcol], F, kind="ExternalInput")
    assert nch >= 3
    partials = nc.dram_tensor(
        "partials", [G * QC, 2 * G * QC], F, kind="ExternalOutput")

    from contextlib import ExitStack

    _es = ExitStack()
    with _es:
        def sb(name, shape, dt):
            return _es.enter_context(nc.sbuf_tensor(name, shape, dt))

        NPB = 3  # pred chunk buffers
        lb = sb("lb", [P, fcol], BF)
        wb = sb("wb", [P, fcol], BF)
        cls = sb("cls", [P, C, QC], BF)
        predc = [sb(f"predc{i}", [P, C, ch], BF) for i in range(NPB)]
        pww = [sb(f"pww{i}", [P, tpc, G, QC], BF) for i in range(2)]
        mmo = [sb(f"mmo{i}", [P, tpc, G, QC], BF) for i in range(2)]

        # chunks 6/7/8 get their masks prebuilt by gpsimd via per-class
        # tensor_scalar is_equal ops (plain tensor_tensor is rejected by the
        # real TRN2 ISA on the Pool engine; tensor_scalar is accepted)
        if nch >= 12 and cws[5] == cws[6] == ch:
            pm = (5, 6, 8)  # chunk 7's mask buffer would overflow SBUF
        elif nch >= 10 and cws[6] == cws[7] == ch:
            pm = (6, 7, 8)
        else:
            pm = ()
        # last two chunks are contiguous and small: fetch them in one DMA
        merge_tail = nch >= 10 and cws[-2] + cws[-1] <= ch
        mmo_pm = {k: sb(f"mmopm{j}", [P, cws[k] // QC, G, QC], BF)
                  for j, k in enumerate(pm)}
        outsb = sb("outsb", [G * QC, 2 * G * QC], F)
        ps = _es.enter_context(nc.psum_tensor("ps", [G * QC, G * QC], F))
        ps2 = _es.enter_context(nc.psum_tensor("ps2", [G * QC, G * QC], F))

        def sem(name):
            return _es.enter_context(nc.semaphore(name))

        l_sem = sem("l_sem")
        lrest_sem = sem("lrest_sem")
        w_sem = sem("w_sem")
        wrest_sem = sem("wrest_sem")
        c0b_sem = sem("c0b_sem")
        c2b_sem = sem("c2b_sem")
        done_sem = sem("done_sem")
        cls_sem = sem("cls_sem")
        ones_sem = sem("ones_sem")
        pdma_sems = [sem(f"pdma_sem{i}") for i in range(NPB)]
        vsem = sem("vsem")
        asem = sem("asem")
        pe_sem = sem("pe_sem")
        pmask_sem = sem("pmask_sem")

        block = _es.enter_context(nc.Block())

        @block.gpsimd
        def _(g: bass.BassEngine):
            # L/W head pieces first (gate chunk-0/1 compute), rests after c1
            g.dma_start(out=lb[:, 0:head], in_=tgt[0][:, 0:head]).then_inc(
                l_sem, 16)
            g.dma_start(out=wb[:, 0:head], in_=tgt[1][:, 0:head]).then_inc(
                w_sem, 16)

            def chunk_dma(k):
                i, cw, o = k % NPB, cws[k], offs[k]
                src = pred.ap()[:, :, o:o + cw].transpose([1, 0, 2])
                g.dma_start(out=predc[i][:, :, 0:cw], in_=src).then_inc(
                    pdma_sems[i], 16)

            # chunk 0 in two class-halves so its pw can start sooner
            cw0 = cws[0]
            src0 = pred.ap()[0:10, :, 0:cw0].transpose([1, 0, 2])
            g.dma_start(out=predc[0][:, 0:10, 0:cw0], in_=src0).then_inc(
                pdma_sems[0], 16)
            src0b = pred.ap()[10:C, :, 0:cw0].transpose([1, 0, 2])
            g.dma_start(out=predc[0][:, 10:C, 0:cw0], in_=src0b).then_inc(
                c0b_sem, 16)
            # tiny class-id constant [P, C, QC]; value = class index
            g.iota(cls[:], pattern=[[1, C], [0, QC]], base=0,
                   channel_multiplier=0,
                   allow_small_or_imprecise_dtypes=True).then_inc(cls_sem, 1)
            chunk_dma(1)
            g.dma_start(out=lb[:, head:], in_=tgt[0][:, head:]).then_inc(
                lrest_sem, 16)
            g.dma_start(out=wb[:, head:], in_=tgt[1][:, head:]).then_inc(
                wrest_sem, 16)
            # chunk 2 (first full-width chunk, at the pipeline-fill boundary)
            # in two class-halves so its pw can start mid-transfer
            cw2, o2 = cws[2], offs[2]
            src2 = pred.ap()[0:10, :, o2:o2 + cw2].transpose([1, 0, 2])
            g.dma_start(out=predc[2][:, 0:10, 0:cw2], in_=src2).then_inc(
                pdma_sems[2], 16)
            src2b = pred.ap()[10:C, :, o2:o2 + cw2].transpose([1, 0, 2])
            g.dma_start(out=predc[2][:, 10:C, 0:cw2], in_=src2b).then_inc(
                c2b_sem, 16)

            pmops = [(kp, c) for kp in pm for c in range(C)]

            def pm_op(kp, c):
                t_k = cws[kp] // QC
                o = offs[kp]
                lsl = lb[:, o:o + cws[kp]].rearrange(
                    "p (t q) -> p t q", q=QC).unsqueeze(2)
                op = g.tensor_scalar(
                    out=mmo_pm[kp][:, 0:t_k, c:c + 1, :], in0=lsl,
                    scalar1=float(c), scalar2=None, op0=is_eq)
                if c == C - 1:
                    op.then_inc(pmask_sem, 1)

            if pm:
                for kp in pm:
                    g.memset(mmo_pm[kp][:, :, C:G, :], 1.0)
                g.wait_ge(lrest_sem, 16)
            qi = 0
            for k in range(3, nch):
                # weave premask ops ahead of each gated wait so the Pool is
                # never the reason a pred DMA stalls
                nbatch = min(len(pmops) - qi, 8)
                for _ in range(nbatch):
                    pm_op(*pmops[qi])
                    qi += 1
                if merge_tail and k == nch - 1:
                    continue  # covered by chunk nch-2's widened DMA
                g.wait_ge(vsem, k - 2)
                if merge_tail and k == nch - 2:
                    i, o = k % NPB, offs[k]
                    cw = cws[k] + cws[k + 1]
                    src = pred.ap()[:, :, o:o + cw].transpose([1, 0, 2])
                    g.dma_start(out=predc[i][:, :, 0:cw], in_=src).then_inc(
                        pdma_sems[i], 16)
                else:
                    chunk_dma(k)
            while qi < len(pmops):
                pm_op(*pmops[qi])
                qi += 1

        @block.vector
        def _(v: bass.BassEngine):
            # ones planes built in DVE's idle startup window
            v.memset(mmo[0][:, :, C:G, :], 1.0).then_inc(ones_sem, 1)
            v.memset(mmo[1][:, :, C:G, :], 1.0).then_inc(ones_sem, 1)
            v.wait_ge(cls_sem, 1)
            v.wait_ge(l_sem, 16)
            for k in range(nch):
                i, ib = k % NPB, k % 2
                cw, o = cws[k], offs[k]
                t_k = cw // QC
                if k == 2:
                    v.wait_ge(lrest_sem, 16)
                    v.wait_ge(wrest_sem, 16)
                if k >= 2:
                    v.wait_ge(pe_sem, k - 1)
                # [P, t_k, 1, QC] views of this chunk's W / L columns
                wslice = wb[:, o:o + cw].rearrange(
                    "p (t q) -> p t q", q=QC).unsqueeze(2)
                lslice = lb[:, o:o + cw].rearrange(
                    "p (t q) -> p t q", q=QC).unsqueeze(2)
                if merge_tail and k == nch - 1:
                    # this chunk rode along in chunk nch-2's widened DMA
                    i = (nch - 2) % NPB
                    kw = nch - 2
                    b0 = cws[nch - 2]
                    pview = predc[i].ap()[:, :, b0:b0 + cw].rearrange(
                        "p c (t q) -> p t c q", q=QC)
                else:
                    kw = k
                    pview = predc[i].ap()[:, :, 0:cw].rearrange(
                        "p c (t q) -> p t c q", q=QC)
                # mask first: it does not depend on this chunk's pred DMA
                if k not in mmo_pm:
                    v.tensor_tensor(
                        out=mmo[ib][:, 0:t_k, 0:C, :],
                        in0=lslice.broadcast_to([P, t_k, C, QC]),
                        in1=cls.ap().unsqueeze(1).broadcast_to(
                            [P, t_k, C, QC]),
                        op=is_eq,
                    )
                if k == 0:
                    v.wait_ge(w_sem, 16)
                v.wait_ge(pdma_sems[i], 16 * (kw // NPB + 1))
                if k in (0, 2):
                    # split chunk: second class-half arrives in its own DMA
                    v.tensor_tensor(
                        out=pww[ib][:, 0:t_k, 0:10, :],
                        in0=predc[i].ap()[:, 0:10, 0:cw].rearrange(
                            "p c (t q) -> p t c q", q=QC),
                        in1=wslice.broadcast_to([P, t_k, 10, QC]), op=mult,
                    )
                    v.wait_ge(c0b_sem if k == 0 else c2b_sem, 16)
                    v.tensor_tensor(
                        out=pww[ib][:, 0:t_k, 10:C, :],
                        in0=predc[i].ap()[:, 10:C, 0:cw].rearrange(
                            "p c (t q) -> p t c q", q=QC),
                        in1=wslice.broadcast_to([P, t_k, C - 10, QC]),
                        op=mult,
                    ).then_inc(vsem, 1)
                else:
                    v.tensor_tensor(
                        out=pww[ib][:, 0:t_k, 0:C, :], in0=pview,
                        in1=wslice.broadcast_to([P, t_k, C, QC]), op=mult,
                    ).then_inc(vsem, 1)
            v.wait_ge(pe_sem, nch - 1)
            v.tensor_copy(out=outsb[:, 0:G * QC], in_=ps[:]).then_inc(vsem, 1)
            v.wait_ge(pe_sem, nch)
            v.tensor_copy(out=outsb[:, G * QC:], in_=ps2[:]).then_inc(vsem, 1)

        @block.scalar
        def _(a: bass.BassEngine):
            # the per-chunk W column copies (pww group 19) run on the
            # otherwise-idle Activation engine
            a.wait_ge(w_sem, 16)
            for k in range(nch):
                ib, cw, o = k % 2, cws[k], offs[k]
                t_k = cw // QC
                if k == 2:
                    a.wait_ge(wrest_sem, 16)
                if k >= 2:
                    a.wait_ge(pe_sem, k - 1)
                wslice = wb[:, o:o + cw].rearrange(
                    "p (t q) -> p t q", q=QC).unsqueeze(2)
                a.copy(out=pww[ib][:, 0:t_k, C:G, :],
                       in_=wslice).then_inc(asem, 1)

        @block.tensor
        def _(t: bass.BassEngine):
            t.wait_ge(ones_sem, 2)
            for k in range(nch):
                i = k % 2
                t_k = cws[k] // QC  # noqa: buffer parity for pww/mmo
                t.wait_ge(vsem, k + 1)
                t.wait_ge(asem, k + 1)
                if k in mmo_pm:
                    t.wait_ge(pmask_sem, 1 + pm.index(k))
                rhs = mmo_pm.get(k, mmo[i])
                last = k == nch - 1
                bank = ps2 if last else ps
                for j in range(t_k):
                    mm = t.matmul(
                        bank[:],
                        pww[i][:, j, :, :],
                        rhs[:, j, :, :],
                        start=(j == 0 and k in (0, nch - 1)),
                        stop=(j == t_k - 1 and k in (nch - 2, nch - 1)),
                    )
                mm.then_inc(pe_sem, 1)

        @block.sync
        def _(s: bass.BassEngine):
            s.wait_ge(vsem, nch + 1)
            s.dma_start(
                out=partials[:, 0:G * QC], in_=outsb[:, 0:G * QC]
            ).then_inc(done_sem, 16)
            s.wait_ge(vsem, nch + 2)
            s.dma_start(
                out=partials[:, G * QC:], in_=outsb[:, G * QC:]
            ).then_inc(done_sem, 16)

    return nc


def _combine(parts: np.ndarray) -> np.ndarray:
    # parts: [ncores, G*QC * 2*G*QC] f32 — two PSUM banks side by side
    tot = parts.astype(np.float64).sum(axis=0).reshape(G * QC, 2, G * QC)
    tot = tot.sum(axis=1).reshape(G, QC, G, QC)
    M = np.einsum("aqbq->ab", tot)
    inter = np.diagonal(M)[:C]
    psum = M[:C, C]
    tsum = M[C, :C]
    dice = (2.0 * inter + SMOOTH) / (psum + tsum + SMOOTH)
    loss = np.sum(1.0 - dice) / C
    return np.asarray(loss, dtype=np.float32)


def kernel(pred: np.ndarray, target: np.ndarray) -> np.ndarray:
    pred = np.asarray(pred, dtype=np.float32)
    target = np.asarray(target, dtype=np.float32)
    B, C_, H, Wd = pred.shape
    fcol = H * Wd // P
    pred_r = np.ascontiguousarray(pred.reshape(B, C_, P, fcol))
    tgt_r = np.ascontiguousarray(target.reshape(B, 2, P, fcol))

    nc = build_nc(fcol)
    in_maps = [{"pred": pred_r[i], "target": tgt_r[i]} for i in range(B)]
    res = run_bass_kernel_spmd(nc, in_maps, list(range(B))).results
    parts = np.stack([r["partials"].reshape(-1) for r in res])
    return _combine(parts)


# revision 4
# speedup vs baseline: 1.0495x; 1.0280x over previous
"""Weighted per-class dice loss on 8 trn2 NeuronCores (batch-sharded), v2.

Per core (one batch element b), pixels viewed as [128, fcol]:
  - SWDGE (gpsimd) casting DMAs bring pred chunks in as bf16 [128, 19, CH].
  - DVE computes, per chunk, two 2x-mode tensor_tensor ops into interleaved
    [P, CH/QC, 20, QC] buffers (QC pixel columns per matmul):
      PW[:, t, c, q]  = pred_c * W       (in1 = W broadcast over classes)
      MM[:, t, c, q]  = (L == c)          (vs a tiny broadcast CLS constant)
    plus a tiny copy of W into PW group 19; MM group 19 = ones.
  - PE accumulates, for each t, one matmul over QC pixel columns
      out[m, n] += sum_p PW[p, t, m] * MM[p, t, n]   (m = g*QC + q)
    into a single [80, 80] PSUM accumulation group:
      out[c, c]   -> inter[c]   (pred_c * W summed where L == c)
      out[c, 19]  -> psum[c]    (pred_c * W summed everywhere)
      out[19, c]  -> tsum[c]    (W summed where L == c)
  - Host folds the QC column-phases + 8 cores and applies the dice formula.
"""

import numpy as np

import concourse.bass as bass
from concourse import mybir
from concourse.bass_utils import run_bass_kernel_spmd

C = 19
G = 20            # 19 classes + 1 (W / ones group)
P = 128
FCOL_FULL = 4096
CH_FULL = 512     # chunk width (pixel columns per chunk)
QC = 4            # pixel columns folded into one matmul
SMOOTH = 1.0

F = mybir.dt.float32
BF = mybir.dt.bfloat16

mult = mybir.AluOpType.mult
is_eq = mybir.AluOpType.is_equal


def chunk_widths(fcol: int, ch: int) -> list[int]:
    """Taper first/last chunks to shrink pipeline fill/drain time."""
    if fcol % ch or ch % (2 * QC):
        return [ch] * max(fcol // ch, 1) if fcol >= ch else [fcol]
    h, q = ch // 2, ch // 4
    if fcol // ch >= 4:
        mid = (fcol - 2 * h - h - 2 * q) // ch
        return [h, h] + [ch] * mid + [h, q, q]
    return [ch] * (fcol // ch)


def build_nc(fcol: int = FCOL_FULL, ch: int = CH_FULL) -> bass.Bass:
    cws = chunk_widths(fcol, ch)
    nch = len(cws)
    offs = [sum(cws[:k]) for k in range(nch)]
    assert sum(cws) == fcol and all(c % QC == 0 for c in cws)
    tpc = ch // QC  # matmuls per full-size chunk (buffer capacity)
    head = cws[0] + cws[1]  # columns covered by the early W/L pieces

    nc = bass.Bass()
    pred = nc.dram_tensor("pred", [C, P, fcol], F, kind="ExternalInput")
    tgt = nc.dram_tensor("target", [2, P, fcol], F, kind="ExternalInput")
    assert nch >= 3
    partials = nc.dram_tensor(
        "partials", [G * QC, 2 * G * QC], F, kind="ExternalOutput")

    from contextlib import ExitStack

    _es = ExitStack()
    with _es:
        def sb(name, shape, dt):
            return _es.enter_context(nc.sbuf_tensor(name, shape, dt))

        NPB = 3  # pred chunk buffers
        lb = sb("lb", [P, fcol], BF)
        wb = sb("wb", [P, fcol], BF)
        cls = sb("cls", [P, C, QC], BF)
        predc = [sb(f"predc{i}", [P, C, ch], BF) for i in range(NPB)]
        pww = [sb(f"pww{i}", [P, tpc, G, QC], BF) for i in range(2)]
        mmo = [sb(f"mmo{i}", [P, tpc, G, QC], BF) for i in range(2)]

        # chunks 6/7/8 get their masks prebuilt by gpsimd via per-class
        # tensor_scalar is_equal ops (plain tensor_tensor is rejected by the
        # real TRN2 ISA on the Pool engine; tensor_scalar is accepted)
        if nch >= 11 and cws[6] == cws[7] == ch:
            pm = (6, 7, 8, 9, 10)  # gpsimd prebuilds these chunks' masks
        elif nch >= 10 and cws[6] == cws[7] == ch:
            pm = (6, 7, 8)
        else:
            pm = ()
        # last two chunks are contiguous and small: fetch them in one DMA
        merge_tail = nch >= 10 and cws[-2] + cws[-1] <= ch
        # tail chunks reuse mmo[k%2] (PE is done reading it by then, since
        # the preceding chunks read premask buffers); others get their own
        mmo_pm = {k: (mmo[k % 2] if k >= nch - 2 else
                      sb(f"mmopm{j}", [P, cws[k] // QC, G, QC], BF))
                  for j, k in enumerate(pm)}
        # pack the last three chunks' pw into pww[0] at t-offsets: one
        # buffer-reuse wait (after PE chunk nch-4) instead of three
        tail_pack = (nch >= 11 and merge_tail
                     and all(k in mmo_pm for k in (nch - 2, nch - 1))
                     and (cws[-3] + cws[-2] + cws[-1]) // QC <= tpc)
        toff = ({nch - 3: 0, nch - 2: cws[-3] // QC,
                 nch - 1: (cws[-3] + cws[-2]) // QC} if tail_pack else {})
        outsb = sb("outsb", [G * QC, 2 * G * QC], F)
        ps = _es.enter_context(nc.psum_tensor("ps", [G * QC, G * QC], F))
        ps2 = _es.enter_context(nc.psum_tensor("ps2", [G * QC, G * QC], F))

        def sem(name):
            return _es.enter_context(nc.semaphore(name))

        l_sem = sem("l_sem")
        lrest_sem = sem("lrest_sem")
        w_sem = sem("w_sem")
        wrest_sem = sem("wrest_sem")
        c0b_sem = sem("c0b_sem")
        c2b_sem = sem("c2b_sem")
        done_sem = sem("done_sem")
        cls_sem = sem("cls_sem")
        ones_sem = sem("ones_sem")
        pdma_sems = [sem(f"pdma_sem{i}") for i in range(NPB)]
        vsem = sem("vsem")
        asem = sem("asem")
        pe_sem = sem("pe_sem")
        pmask_sem = sem("pmask_sem")

        block = _es.enter_context(nc.Block())

        @block.gpsimd
        def _(g: bass.BassEngine):
            # L/W head pieces first (gate chunk-0/1 compute), rests after c1
            g.dma_start(out=lb[:, 0:head], in_=tgt[0][:, 0:head]).then_inc(
                l_sem, 16)
            g.dma_start(out=wb[:, 0:head], in_=tgt[1][:, 0:head]).then_inc(
                w_sem, 16)

            def chunk_dma(k):
                i, cw, o = k % NPB, cws[k], offs[k]
                src = pred.ap()[:, :, o:o + cw].transpose([1, 0, 2])
                g.dma_start(out=predc[i][:, :, 0:cw], in_=src).then_inc(
                    pdma_sems[i], 16)

            # chunk 0 in two class-halves so its pw can start sooner
            cw0 = cws[0]
            src0 = pred.ap()[0:10, :, 0:cw0].transpose([1, 0, 2])
            g.dma_start(out=predc[0][:, 0:10, 0:cw0], in_=src0).then_inc(
                pdma_sems[0], 16)
            src0b = pred.ap()[10:C, :, 0:cw0].transpose([1, 0, 2])
            g.dma_start(out=predc[0][:, 10:C, 0:cw0], in_=src0b).then_inc(
                c0b_sem, 16)
            # tiny class-id constant [P, C, QC]; value = class index
            g.iota(cls[:], pattern=[[1, C], [0, QC]], base=0,
                   channel_multiplier=0,
                   allow_small_or_imprecise_dtypes=True).then_inc(cls_sem, 1)
            chunk_dma(1)
            g.dma_start(out=lb[:, head:], in_=tgt[0][:, head:]).then_inc(
                lrest_sem, 16)
            g.dma_start(out=wb[:, head:], in_=tgt[1][:, head:]).then_inc(
                wrest_sem, 16)
            # chunk 2 (first full-width chunk, at the pipeline-fill boundary)
            # in two class-halves so its pw can start mid-transfer
            cw2, o2 = cws[2], offs[2]
            src2 = pred.ap()[0:10, :, o2:o2 + cw2].transpose([1, 0, 2])
            g.dma_start(out=predc[2][:, 0:10, 0:cw2], in_=src2).then_inc(
                pdma_sems[2], 16)
            src2b = pred.ap()[10:C, :, o2:o2 + cw2].transpose([1, 0, 2])
            g.dma_start(out=predc[2][:, 10:C, 0:cw2], in_=src2b).then_inc(
                c2b_sem, 16)

            pmops = [(kp, c) for kp in pm for c in range(C)]

            def pm_op(kp, c):
                if c == 0 and kp >= nch - 2:
                    # reusing mmo[kp%2]: wait until PE's last read of it
                    # (chunk kp-4; chunks kp-2/kp-3 read premask buffers)
                    g.wait_ge(pe_sem, kp - 3)
                t_k = cws[kp] // QC
                o = offs[kp]
                lsl = lb[:, o:o + cws[kp]].rearrange(
                    "p (t q) -> p t q", q=QC).unsqueeze(2)
                op = g.tensor_scalar(
                    out=mmo_pm[kp][:, 0:t_k, c:c + 1, :], in0=lsl,
                    scalar1=float(c), scalar2=None, op0=is_eq)
                if c == C - 1:
                    op.then_inc(pmask_sem, 1)

            if pm:
                for kp in pm:
                    if kp < nch - 2:
                        g.memset(mmo_pm[kp][:, :, C:G, :], 1.0)
                g.wait_ge(lrest_sem, 16)
            qi = 0
            for k in range(3, nch):
                # weave premask ops ahead of each gated wait so the Pool is
                # never the reason a pred DMA stalls
                nbatch = min(len(pmops) - qi, 8)
                for _ in range(nbatch):
                    pm_op(*pmops[qi])
                    qi += 1
                if merge_tail and k == nch - 1:
                    continue  # covered by chunk nch-2's widened DMA
                g.wait_ge(vsem, k - 2)
                if merge_tail and k == nch - 2:
                    i, o = k % NPB, offs[k]
                    cw = cws[k] + cws[k + 1]
                    src = pred.ap()[:, :, o:o + cw].transpose([1, 0, 2])
                    g.dma_start(out=predc[i][:, :, 0:cw], in_=src).then_inc(
                        pdma_sems[i], 16)
                else:
                    chunk_dma(k)
            while qi < len(pmops):
                pm_op(*pmops[qi])
                qi += 1

        @block.vector
        def _(v: bass.BassEngine):
            # ones planes built in DVE's idle startup window
            v.memset(mmo[0][:, :, C:G, :], 1.0).then_inc(ones_sem, 1)
            v.memset(mmo[1][:, :, C:G, :], 1.0).then_inc(ones_sem, 1)
            v.wait_ge(cls_sem, 1)
            v.wait_ge(l_sem, 16)
            for k in range(nch):
                i, ib = k % NPB, k % 2
                cw, o = cws[k], offs[k]
                t_k = cw // QC
                if k == 2:
                    v.wait_ge(lrest_sem, 16)
                    v.wait_ge(wrest_sem, 16)
                if k >= 2 and not (tail_pack and k > nch - 3):
                    v.wait_ge(pe_sem, k - 1)
                pb, to = ((pww[0], toff[k]) if k in toff
                          else (pww[ib], 0))
                # [P, t_k, 1, QC] views of this chunk's W / L columns
                wslice = wb[:, o:o + cw].rearrange(
                    "p (t q) -> p t q", q=QC).unsqueeze(2)
                lslice = lb[:, o:o + cw].rearrange(
                    "p (t q) -> p t q", q=QC).unsqueeze(2)
                if merge_tail and k == nch - 1:
                    # this chunk rode along in chunk nch-2's widened DMA
                    i = (nch - 2) % NPB
                    kw = nch - 2
                    b0 = cws[nch - 2]
                    pview = predc[i].ap()[:, :, b0:b0 + cw].rearrange(
                        "p c (t q) -> p t c q", q=QC)
                else:
                    kw = k
                    pview = predc[i].ap()[:, :, 0:cw].rearrange(
                        "p c (t q) -> p t c q", q=QC)
                # mask first: it does not depend on this chunk's pred DMA
                if k not in mmo_pm:
                    v.tensor_tensor(
                        out=mmo[ib][:, 0:t_k, 0:C, :],
                        in0=lslice.broadcast_to([P, t_k, C, QC]),
                        in1=cls.ap().unsqueeze(1).broadcast_to(
                            [P, t_k, C, QC]),
                        op=is_eq,
                    )
                if k == 0:
                    v.wait_ge(w_sem, 16)
                v.wait_ge(pdma_sems[i], 16 * (kw // NPB + 1))
                if k in (0, 2):
                    # split chunk: second class-half arrives in its own DMA
                    v.tensor_tensor(
                        out=pb[:, to:to + t_k, 0:10, :],
                        in0=predc[i].ap()[:, 0:10, 0:cw].rearrange(
                            "p c (t q) -> p t c q", q=QC),
                        in1=wslice.broadcast_to([P, t_k, 10, QC]), op=mult,
                    )
                    v.wait_ge(c0b_sem if k == 0 else c2b_sem, 16)
                    v.tensor_tensor(
                        out=pb[:, to:to + t_k, 10:C, :],
                        in0=predc[i].ap()[:, 10:C, 0:cw].rearrange(
                            "p c (t q) -> p t c q", q=QC),
                        in1=wslice.broadcast_to([P, t_k, C - 10, QC]),
                        op=mult,
                    ).then_inc(vsem, 1)
                else:
                    v.tensor_tensor(
                        out=pb[:, to:to + t_k, 0:C, :], in0=pview,
                        in1=wslice.broadcast_to([P, t_k, C, QC]), op=mult,
                    ).then_inc(vsem, 1)
            v.wait_ge(pe_sem, nch - 2)
            v.tensor_copy(out=outsb[:, 0:G * QC], in_=ps[:]).then_inc(vsem, 1)
            v.wait_ge(pe_sem, nch)
            v.tensor_copy(out=outsb[:, G * QC:], in_=ps2[:]).then_inc(vsem, 1)

        @block.scalar
        def _(a: bass.BassEngine):
            # the per-chunk W column copies (pww group 19) run on the
            # otherwise-idle Activation engine
            a.wait_ge(w_sem, 16)
            for k in range(nch):
                ib, cw, o = k % 2, cws[k], offs[k]
                t_k = cw // QC
                if k == 2:
                    a.wait_ge(wrest_sem, 16)
                if k >= 2 and not (tail_pack and k > nch - 3):
                    a.wait_ge(pe_sem, k - 1)
                pb, to = ((pww[0], toff[k]) if k in toff
                          else (pww[ib], 0))
                wslice = wb[:, o:o + cw].rearrange(
                    "p (t q) -> p t q", q=QC).unsqueeze(2)
                a.copy(out=pb[:, to:to + t_k, C:G, :],
                       in_=wslice).then_inc(asem, 1)

        @block.tensor
        def _(t: bass.BassEngine):
            t.wait_ge(ones_sem, 2)
            for k in range(nch):
                i = k % 2
                t_k = cws[k] // QC  # noqa: buffer parity for pww/mmo
                t.wait_ge(vsem, k + 1)
                t.wait_ge(asem, k + 1)
                if k in mmo_pm:
                    t.wait_ge(pmask_sem, 1 + pm.index(k))
                rhs = mmo_pm.get(k, mmo[i])
                bank = ps2 if k >= nch - 2 else ps
                lb_, lo = ((pww[0], toff[k]) if k in toff
                           else (pww[i], 0))
                for j in range(t_k):
                    mm = t.matmul(
                        bank[:],
                        lb_[:, lo + j, :, :],
                        rhs[:, j, :, :],
                        start=(j == 0 and k in (0, nch - 2)),
                        stop=(j == t_k - 1 and k in (nch - 3, nch - 1)),
                    )
                mm.then_inc(pe_sem, 1)

        @block.sync
        def _(s: bass.BassEngine):
            s.wait_ge(vsem, nch + 1)
            s.dma_start(
                out=partials[:, 0:G * QC], in_=outsb[:, 0:G * QC]
            ).then_inc(done_sem, 16)
            s.wait_ge(vsem, nch + 2)
            s.dma_start(
                out=partials[:, G * QC:], in_=outsb[:, G * QC:]
            ).then_inc(done_sem, 16)

    return nc


def _combine(parts: np.ndarray) -> np.ndarray:
    # parts: [ncores, G*QC * 2*G*QC] f32 — two PSUM banks side by side
    tot = parts.astype(np.float64).sum(axis=0).reshape(G * QC, 2, G * QC)
    tot = tot.sum(axis=1).reshape(G, QC, G, QC)
    M = np.einsum("aqbq->ab", tot)
    inter = np.diagonal(M)[:C]
    psum = M[:C, C]
    tsum = M[C, :C]
    dice = (2.0 * inter + SMOOTH) / (psum + tsum + SMOOTH)
    loss = np.sum(1.0 - dice) / C
    return np.asarray(loss, dtype=np.float32)


def kernel(pred: np.ndarray, target: np.ndarray) -> np.ndarray:
    pred = np.asarray(pred, dtype=np.float32)
    target = np.asarray(target, dtype=np.float32)
    B, C_, H, Wd = pred.shape
    fcol = H * Wd // P
    pred_r = np.ascontiguousarray(pred.reshape(B, C_, P, fcol))
    tgt_r = np.ascontiguousarray(target.reshape(B, 2, P, fcol))

    nc = build_nc(fcol)
    in_maps = [{"pred": pred_r[i], "target": tgt_r[i]} for i in range(B)]
    res = run_bass_kernel_spmd(nc, in_maps, list(range(B))).results
    parts = np.stack([r["partials"].reshape(-1) for r in res])
    return _combine(parts)
